# revision 28
# baseline (speedup 1.0000x reference)
"""HGNN conv kernel for Trainium2, data-parallel over time across 8 cores.

Per core (t = core index): out_b = Dv^-1/2 Gc De^-1 Gc^T Dv^-1/2 (x_b W + 1 b^T)
computed in factored form (L never materialized):
  Gs  = Dv^-1/2 Gc                      [N, E]
  z   = x_t^T Gs  per 128-row bf block  [BF, E]   (MM1, bf16, k-outer passes)
  v   = z-blocks^T @ blockdiag(W,W) + u0 bias^T   [E, BF]  (W-MM, bf16)
  out = Gsd^T v with Gsd = de * Gs^T    [N, BF]   (MM2, bf16)

Two node orderings, bridged at the Gs transpose:
 - Contraction side (x, Gc, Gs): k-order, n = 128*k + p. Every matmul
   operand is a contiguous [128, width] slice (the BIR verifier allows only
   one free dimension). x is cast f32->bf16 in the DMA, which also halves
   the descriptor payload.
 - Store side (Gsd columns = MM2 output rows): (s, t2)-order,
   n = 256*s + 2*p + t2, so each output-store descriptor covers the
   (t2, f) 512B contiguous run of two consecutive HBM rows -- full DMA
   rate instead of the 2x small-descriptor penalty.
   The transpose evictions read tp columns with stride 2 (q = 2c + t2) to
   land Gsd directly in store order.

Schedule: Gc (f32r cast DMAs) first, then x in three b-groups matched to
three k-outer MM1 passes (5/5/4 m-tiles, one PSUM bank each); per-k-half
dv chains chase the Gc DMAs; each pass's W-MM unlocks one MM2 wave so
stores stream through the whole back half. A chain of dummy transposes
holds the PE clock warm until real work arrives. PSUM budget (8 banks):
5 z + 1 (warmup/stats/W-MM) + 2 (statsT/transposes/MM2).
"""

import sys

import numpy as np

sys.path.insert(0, "/opt/trn_rl_repo")

from contextlib import ExitStack

import concourse.bass as bass
import concourse.mybir as mybir
import concourse.tile as tile
from concourse import bacc, bass_utils
from concourse.masks import make_identity

P = 128
T = 8
B = 28          # batch entries per core
N = 1024        # nodes
E = 512         # hyperedges (256 static + 256 dynamic)
F = 64          # features
BF = B * F      # 1792
EPS = 1e-6
NT = 8          # n k-tiles (contraction side)
NS = 4          # n supertiles (store side: 256 nodes, 2 per partition)
T2 = 2          # consecutive n rows per partition (store side)
ET = E // P     # 4 e-tiles
MT = BF // P    # 14 bf-tiles (2 batch entries each)
NB = 4          # output free-dim chunks
NBW = BF // NB  # 448 = 7 batch entries * 64
BC = B // NB    # 7 batch entries per store chunk
PASS_M = ((0, 5), (5, 10), (10, 14))  # m-tile ranges per MM1 pass
N_WARM = 33     # dummy transposes to hold the PE clock warm until stats

f32 = mybir.dt.float32
f32r = mybir.dt.float32r
bf16 = mybir.dt.bfloat16


def _build_nc():
    nc = bacc.Bacc("TRN2", target_bir_lowering=False, debug=False)

    xs = nc.dram_tensor("xs", [B, N, F], f32, kind="ExternalInput").ap()
    g = nc.dram_tensor("g", [N, 256], f32, kind="ExternalInput").ap()
    g1 = nc.dram_tensor("g1", [N, 256], f32, kind="ExternalInput").ap()
    w = nc.dram_tensor("w", [F, F], f32, kind="ExternalInput").ap()
    bvec = nc.dram_tensor("b", [F], f32, kind="ExternalInput").ap()
    os_ = nc.dram_tensor("os", [B, N, F], f32, kind="ExternalOutput").ap()

    with tile.TileContext(nc) as tc, ExitStack() as ctx:
        const = ctx.enter_context(tc.tile_pool(name="const", bufs=1))
        big = ctx.enter_context(tc.tile_pool(name="big", bufs=1))
        ztp = ctx.enter_context(tc.tile_pool(name="ztp", bufs=3))
        osb = ctx.enter_context(tc.tile_pool(name="osb", bufs=4))
        ps_a = ctx.enter_context(tc.tile_pool(name="ps_a", bufs=1, space="PSUM"))
        ps_b = ctx.enter_context(tc.tile_pool(name="ps_b", bufs=2, space="PSUM"))
        ps_z = ctx.enter_context(tc.tile_pool(name="ps_z", bufs=1, space="PSUM"))

        # ---- input loads -------------------------------------------------
        # Gc = [G | G1] in k-order, f32r cast in the (SWDGE) DMA; two
        # k-halves each so the dv chains can chase.
        gcg = big.tile([P, NT, 256], f32r, name="gcg")
        gcg1 = big.tile([P, NT, 256], f32r, name="gcg1")
        g_r = g.rearrange("(k p) e -> p k e", p=P)
        g1_r = g1.rearrange("(k p) e -> p k e", p=P)
        for kh in (slice(0, 4), slice(4, 8)):
            nc.gpsimd.dma_start(gcg[:, kh], g_r[:, kh])
            nc.gpsimd.dma_start(gcg1[:, kh], g1_r[:, kh])

        # x in k-order, bf16 cast in the DMA: [p, k, b, f] so MM1 operands
        # are contiguous [128, 128] slices. Three b-groups matching the
        # MM1 passes, k ascending within each group.
        xs_bf = big.tile([P, NT, B, F], bf16, name="xs_bf")
        xs_r = xs.rearrange("b (k p) f -> p k b f", p=P)
        for bsl in (slice(0, 14), slice(14, B)):
            for k in range(NT):
                nc.gpsimd.dma_start(xs_bf[:, k, bsl], xs_r[:, k, bsl])

        # zeros tile for the PE warmup chain (ready before make_identity)
        zz = const.tile([P, P], f32, name="zz")
        nc.vector.memset(zz[:], 0.0)

        ident_f = const.tile([P, P], f32, name="ident_f")
        make_identity(nc, ident_f[:])

        # small constants on HWDGE (qSP carries only these + stores now
        # that Gc/x ride SWDGE)
        bdw_f = const.tile([P, P], f32, name="bdw_f")
        nc.vector.memset(bdw_f[:], 0.0)
        nc.sync.dma_start(bdw_f[0:64, 0:64], w)
        nc.sync.dma_start(bdw_f[64:128, 64:128], w)
        btmp = const.tile([1, F], f32, name="btmp")
        nc.sync.dma_start(btmp[:], bvec[None, :])

        # ---- PE clock warmup while Gc streams in -------------------------
        for _ in range(N_WARM):
            warm = ps_a.tile([P, P], f32, name="warm", tag="a")
            nc.tensor.matmul(warm[:], zz[:], zz[:], is_transpose=True)

        # ---- degree stats ------------------------------------------------
        # dv = 1/sqrt(rowsum(Gc) + eps), chained per k-half so each chases
        # its own G/G1 DMA.
        rsa = const.tile([P, NT], f32, name="rsa")
        rsb = const.tile([P, NT], f32, name="rsb")
        rscr = const.tile([P, 2, 256], f32, name="rscr")
        rs = const.tile([P, NT], f32, name="rs")
        sq = const.tile([P, NT], f32, name="sq")
        eps_col = const.tile([P, 1], f32, name="eps_col")
        nc.vector.memset(eps_col[:], EPS)
        # onesdv_f[..., 0] = 1, onesdv_f[..., 1] = dv; onesdv = rounded f32r
        onesdv_f = const.tile([P, NT, 2], f32, name="onesdv_f")
        nc.vector.memset(onesdv_f[:, :, 0:1], 1.0)
        onesdv = const.tile([P, NT, 2], f32r, name="onesdv")
        for kh in (slice(0, 4), slice(4, 8)):
            for k in range(kh.start, kh.stop):
                # rowsums via Act accum_out (scratch out, sum lands in rsa/rsb)
                nc.scalar.activation(
                    rscr[:, 0], gcg[:, k, :], mybir.ActivationFunctionType.Copy,
                    accum_out=rsa[:, k : k + 1],
                )
                nc.scalar.activation(
                    rscr[:, 1], gcg1[:, k, :], mybir.ActivationFunctionType.Copy,
                    accum_out=rsb[:, k : k + 1],
                )
            nc.vector.tensor_tensor(
                out=rs[:, kh], in0=rsa[:, kh], in1=rsb[:, kh],
                op=mybir.AluOpType.add,
            )
            nc.scalar.activation(
                sq[:, kh], rs[:, kh], mybir.ActivationFunctionType.Sqrt,
                bias=eps_col[:],
            )
            nc.vector.reciprocal(onesdv_f[:, kh, 1:2], sq[:, kh, None])
            nc.vector.tensor_copy(onesdv[:, kh], onesdv_f[:, kh])

        # ---- Gs in bf16 (feeds MM1 rhs and the transposes) ---------------
        gs_bf = big.tile([P, NT, E], bf16, name="gs_bf")
        for k in range(NT):
            nc.vector.tensor_scalar(
                out=gs_bf[:, k, 0:256], in0=gcg[:, k, :],
                scalar1=onesdv_f[:, k, 1:2], scalar2=None,
                op0=mybir.AluOpType.mult,
            )
            nc.vector.tensor_scalar(
                out=gs_bf[:, k, 256:512], in0=gcg1[:, k, :],
                scalar1=onesdv_f[:, k, 1:2], scalar2=None,
                op0=mybir.AluOpType.mult,
            )

        # const processing off the critical dv/gs chain (the scheduler
        # ignores program order; keep late-landing DMA consumers off DVE)
        ident = const.tile([P, P], f32r, name="ident")
        nc.vector.tensor_copy(ident[:], ident_f[:])
        ident_bf = const.tile([P, P], bf16, name="ident_bf")
        nc.vector.tensor_copy(ident_bf[:], ident_f[:])

        # blockdiag(W, W) in bf16 -- cast on the idle Pool engine (the w
        # DMA can land late; Pool has nothing queued then)
        bdw = const.tile([P, P], bf16, name="bdw")
        nc.gpsimd.tensor_copy(bdw[:], bdw_f[:])

        # bias tiled twice [128, 128] f32r (row-broadcast across partitions)
        bias2 = const.tile([1, 2, F], f32r, name="bias2")
        nc.vector.tensor_copy(bias2[:], btmp[0:1, None, :].to_broadcast([1, 2, F]))
        bias_bc = const.tile([P, P], f32r, name="bias_bc")
        nc.gpsimd.partition_broadcast(
            bias_bc[:], bias2[:].rearrange("o t f -> o (t f)")
        )

        gsd_all = big.tile([P, ET, NS, T2, P], bf16, name="gsd_all")
        v_all = big.tile([P, ET, BF], bf16, name="v_all")

        # ---- stats matmul + transposes -----------------------------------
        stats_ps = ps_a.tile([2, E], f32, name="stats_ps", tag="a")
        for k in range(NT):
            nc.tensor.matmul(
                stats_ps[:, 0:256], onesdv[:, k, :], gcg[:, k, :],
                start=(k == 0), stop=(k == NT - 1),
            )
            nc.tensor.matmul(
                stats_ps[:, 256:512], onesdv[:, k, :], gcg1[:, k, :],
                start=(k == 0), stop=(k == NT - 1),
            )
        stats_sb = const.tile([2, E], f32r, name="stats_sb")
        nc.vector.tensor_copy(stats_sb[:], stats_ps[:])

        # transpose stats to column layout [128, ET, 2] = [cs | u0]
        statsT = const.tile([P, ET, 2], f32, name="statsT")
        for j in range(ET):
            tp2 = ps_b.tile([P, P], f32r, name="tp2", tag="b")[:, 0:2]
            nc.tensor.matmul(
                tp2[:], stats_sb[:, j * P : (j + 1) * P], ident[0:2, 0:2],
                is_transpose=True,
            )
            nc.vector.tensor_copy(statsT[:, j, :], tp2[:])
        de_col = const.tile([P, ET], f32, name="de_col")
        nc.vector.tensor_scalar(
            out=de_col[:], in0=statsT[:, :, 0], scalar1=EPS, scalar2=None,
            op0=mybir.AluOpType.add,
        )
        nc.vector.reciprocal(de_col[:], de_col[:])

        # Gsd[e, n] = de[e] * Gs[n, e] via bf16 PE transpose; the strided
        # evictions land Gsd in store order: tp col q of k-tile k=2s+hi is
        # n = 128k + q = 256s + 2c + t2 with t2 = q % 2, c = q//2 + 64*hi.
        for j in range(ET):
            for k in range(NT):
                s, hi = divmod(k, 2)
                tp = ps_b.tile([P, P], bf16, name="tp", tag="b")
                nc.tensor.matmul(
                    tp[:], gs_bf[:, k, j * P : (j + 1) * P],
                    ident_bf[:], is_transpose=True,
                )
                tp_v = tp[:].rearrange("p (c t2) -> p c t2", t2=T2)
                for t2 in range(T2):
                    # scaled copy on Act: out = tp * de  (keeps DVE free)
                    nc.scalar.activation(
                        gsd_all[:, j, s, t2, hi * 64 : (hi + 1) * 64],
                        tp_v[:, :, t2],
                        mybir.ActivationFunctionType.Copy,
                        scale=de_col[:, j : j + 1],
                    )

        # ---- MM2 wave helper ---------------------------------------------
        os_r = os_.rearrange("b (s p t2) f -> p s b t2 f", p=P, t2=T2)

        def mm2_wave(nb):
            for s in range(NS):
                ot = osb.tile([P, BC, T2, F], f32, name="ot")
                for t2 in range(T2):
                    ops = ps_b.tile([P, NBW], f32, name="ops", tag="b")
                    for j in range(ET):
                        nc.tensor.matmul(
                            ops[:], gsd_all[:, j, s, t2, :],
                            v_all[:, j, nb * NBW : (nb + 1) * NBW],
                            start=(j == 0), stop=(j == ET - 1),
                        )
                    src = ops[:].rearrange("p (c f) -> p c f", f=F)
                    if (s + t2) % 2 == 0:
                        nc.scalar.copy(ot[:, :, t2, :], src)
                    else:
                        nc.vector.tensor_copy(ot[:, :, t2, :], src)
                nc.sync.dma_start(os_r[:, s, nb * BC : (nb + 1) * BC, :, :], ot[:])

        # ---- MM1 + W-MM + interleaved MM2 waves --------------------------
        def wmm(mi, zsrc, wtag):
            # W-MM per finished m-tile: v = zt^T-blocks @ bdw + u0 bias^T.
            # wps reuses the PSUM slot the zt copy just freed (same tag), so
            # the per-m W-MM chains run parallel across 5 banks.
            zt = ztp.tile([P, E], bf16, name="zt")
            nc.scalar.copy(zt[:], zsrc[:])
            wps = ps_z.tile([P, ET, P], f32, name="wps", tag=wtag)
            for j in range(ET):
                nc.tensor.matmul(
                    wps[:, j, :], zt[:, j * P : (j + 1) * P], bdw[:],
                    start=True, stop=True,
                )
            for j in range(ET):
                # v = (bias_bcast * u0_col) + zw_psum, rounded to bf16
                nc.vector.scalar_tensor_tensor(
                    out=v_all[:, j, mi * P : (mi + 1) * P],
                    in0=bias_bc[:],
                    scalar=statsT[:, j, 1:2],
                    in1=wps[:, j, :],
                    op0=mybir.AluOpType.mult,
                    op1=mybir.AluOpType.add,
                )

        def mm1(zdst, mi, k):
            b0 = 2 * mi
            nc.tensor.matmul(
                zdst[:],
                xs_bf[:, k, b0 : b0 + 2, :],
                gs_bf[:, k, :],
                start=(k == 0), stop=(k == NT - 1),
            )

        # k-outer passes across 5 PSUM banks, chasing the x stream; each
        # pass's W-MM unlocks one MM2 wave (nb=2 and 3 after the last)
        for pi, (m0, m1) in enumerate(PASS_M):
            zps = [
                ps_z.tile([P, E], f32, name=f"zps{mi}", tag=f"zps{mi - m0}")
                for mi in range(m0, m1)
            ]
            for k in range(NT):
                for mi in range(m0, m1):
                    mm1(zps[mi - m0], mi, k)
            for mi in range(m0, m1):
                wmm(mi, zps[mi - m0], f"zps{mi - m0}")
            mm2_wave(pi)
        mm2_wave(3)

    nc.finalize()
    return nc


_NC = None


def _get_nc():
    global _NC
    if _NC is None:
        _NC = _build_nc()
    return _NC


def kernel(x, G, G1, weight, bias):
    nc = _get_nc()
    x = np.ascontiguousarray(x, dtype=np.float32)
    G = np.ascontiguousarray(G, dtype=np.float32)
    G1 = np.ascontiguousarray(G1, dtype=np.float32)
    weight = np.ascontiguousarray(weight, dtype=np.float32)
    bias = np.ascontiguousarray(bias, dtype=np.float32)

    in_maps = []
    for c in range(T):
        in_maps.append(
            {
                "xs": x[c * B : (c + 1) * B],
                "g": G,
                "g1": np.ascontiguousarray(G1[c]),
                "w": weight,
                "b": bias,
            }
        )
    res = bass_utils.run_bass_kernel_spmd(nc, in_maps, core_ids=list(range(T)))
    return np.concatenate([r["os"] for r in res.results], axis=0)
